# revision 17
# baseline (speedup 1.0000x reference)
"""Multi-head causal attention (B=2, S=2048, D=1024, H=16) on 8 TRN2 NeuronCores.

Sharding: batch x head-group.  Core i handles batch b = i//4 and head-group
hg = i%4 (4 heads = 256 projection columns).  Each core computes
  Q^T/K^T/V = proj(X_b) for its 256 columns, causal attention for its 4
  heads, and a partial output  ctx_slice @ Wo[256-row slice]  ->
  [2048, 1024] fp32 partial.  Host sums the 4 partials per batch and adds bo
  (standard tensor-parallel row-sharded out-projection unshard).

On-core algorithm (per core):
  - X^T tiles built once via PE transposes (fp16).
  - Q^T = Wq^T X^T-route: lhsT=Wq chunk, rhs=X^T  -> Q^T [cols, tok] (fp16)
  - scores computed TRANSPOSED: S^T[k, q] = K @ Q^T via lhsT=K^T slice,
    rhs=Q^T, so softmax's k-reduction lands on the partition axis where the
    PE (ones-column trick) does it for free.
  - softmax without row-max: scores are bounded (|s| < 70 << 88), exp in
    fp32 cannot overflow. P = exp(S^T) in bf16.
  - PV: lhsT = [V | 1] (ones col appended), rhs = P^T -> ctx~^T and the
    softmax denominator in one accumulation.  Normalize with
    reciprocal + gpsimd partition_broadcast + DVE multiply.
  - out-proj: lhsT = ctx^T (already transposed!), rhs = Wo rows (bf16).
Pre-softmax chain runs in fp16 (1 cyc/row on PE, 8x finer mantissa than
bf16), post-softmax in bf16.  Measured end-to-end rel err ~4e-3.
"""

import numpy as np

import concourse.bass as bass
import concourse.mybir as mybir
import concourse.tile as tile
from concourse import bacc
from concourse.bass_utils import run_bass_kernel_spmd
from concourse.masks import make_identity

F32 = mybir.dt.float32
F16 = mybir.dt.float16
BF16 = mybir.dt.bfloat16
AF = mybir.ActivationFunctionType

B, S, D = 2, 2048, 1024
H, HD = 16, 64
NCORES = 8
CG = 256            # projection columns per core (4 heads)
HG_HEADS = 4        # heads per core
TOK_TILES = S // 128   # 16
D_CHUNKS = D // 128    # 8
QH = 2              # q halves of 1024
QHW = 1024          # q-half width
KT = S // 128       # 16 k tiles


def _build_program():
    nc = bacc.Bacc("TRN2", target_bir_lowering=False, debug=False)

    x_d = nc.dram_tensor("X", [S, D], F32, kind="ExternalInput").ap()
    wq_d = nc.dram_tensor("Wq", [D, CG], F32, kind="ExternalInput").ap()
    wk_d = nc.dram_tensor("Wk", [D, CG], F32, kind="ExternalInput").ap()
    wv_d = nc.dram_tensor("Wv", [D, CG], F32, kind="ExternalInput").ap()
    wo_d = nc.dram_tensor("Wo", [CG, D], F32, kind="ExternalInput").ap()
    out_d = nc.dram_tensor("out", [S, D], F32, kind="ExternalOutput").ap()

    with tile.TileContext(nc) as tc:
        _emit(nc, tc, x_d, wq_d, wk_d, wv_d, wo_d, out_d)
    nc.compile()
    return nc


def _emit(nc, tc, x_d, wq_d, wk_d, wv_d, wo_d, out_d):
    with (
        tc.sbuf_pool(name="persist", bufs=1) as pp,
        tc.sbuf_pool(name="work", bufs=1) as wp,
    ):
        # ---- persistent SBUF tensors
        xt = pp.tile([128, D_CHUNKS, S], F16, name="xt")        # X^T  [dval, dchunk, tok]
        qt = pp.tile([128, 2, S], F16, name="qt")               # Q^T  [col, coltile, tok]
        kt = pp.tile([128, 2, S], F16, name="kt")
        vt = pp.tile([128, TOK_TILES, HG_HEADS * 128], BF16, name="vt")  # [1|pad|V] per head
        ctxT = pp.tile([128, 2, S], BF16, name="ctxT")
        wqh = pp.tile([128, D_CHUNKS, CG], F16, name="wqh")
        wkh = pp.tile([128, D_CHUNKS, CG], F16, name="wkh")
        wvh = pp.tile([128, D_CHUNKS, CG], F16, name="wvh")
        wob = pp.tile([128, 2, D], BF16, name="wob")
        ident = pp.tile([128, 128], F16, name="ident")
        cmask = pp.tile([128, 128], BF16, name="cmask")

        # ---- constants
        make_identity(nc, ident)
        # causal 0/1 mask for the diagonal 128x128 block of S^T[k, q]:
        # keep (1.0) where q >= k i.e. col >= partition.
        nc.gpsimd.memset(cmask, 1.0)
        nc.gpsimd.affine_select(
            out=cmask, in_=cmask, compare_op=mybir.AluOpType.is_ge,
            fill=0.0, base=0, pattern=[[1, 128]], channel_multiplier=-1,
        )
        # ones column of [1|pad|V] (ones FIRST so the softmax denominator
        # lands in PSUM row 0 — custom-DVE reciprocal needs a partition-0
        # input; V starts at col 64 so ctx rows are 32-aligned)
        ones_cols = vt.rearrange("p t (h c) -> p t h c", h=HG_HEADS)[:, :, :, 0:1]
        nc.gpsimd.memset(ones_cols, 1.0)

        # ---- X loads issued first so the transpose pipeline starts ASAP
        xs_tiles = []
        for t in range(TOK_TILES):
            xs = wp.tile([128, D], F32, tag="xs", bufs=4, name=f"xs{t}")
            nc.sync.dma_start(xs, x_d[t * 128:(t + 1) * 128, :])
            xs_tiles.append(xs)

        # ---- load + convert weights
        for w_dram, w_sb in ((wq_d, wqh), (wk_d, wkh), (wv_d, wvh)):
            wstage = wp.tile([128, D_CHUNKS, CG], F32, tag="wstage", bufs=2)
            nc.sync.dma_start(wstage, w_dram.rearrange("(dc p) c -> p dc c", p=128))
            nc.vector.tensor_copy(w_sb, wstage)
        wostage = wp.tile([128, 2, D], F32, tag="wstage", bufs=2)
        nc.sync.dma_start(wostage, wo_d.rearrange("(ct p) n -> p ct n", p=128))
        nc.vector.tensor_copy(wob, wostage)

        with tc.psum_pool(name="pp1", bufs=1) as pq:
            # ---- X fp16 convert, xbar DMA-transpose into xt
            # (dma_start_transpose [128,1024] -> [128, 8, 128] writes
            #  out[:, d, :] = X^T rows [128d, 128d+128) — verified on HW)
            for t in range(TOK_TILES):
                xh = wp.tile([128, D], F16, tag="xh", bufs=3)
                nc.vector.tensor_copy(xh, xs_tiles[t])
                nc.sync.dma_start_transpose(
                    xt[:, :, t * 128:(t + 1) * 128], xh)

            # ---- V  (lhsT = X^T tile, rhs = Wv)
            for t in range(TOK_TILES):
                psv = pq.tile([128, CG], F32, tag="vps", bufs=2)
                for d in range(D_CHUNKS):
                    nc.tensor.matmul(
                        psv,
                        lhsT=xt[:, d, t * 128:(t + 1) * 128],
                        rhs=wvh[:, d, :],
                        start=(d == 0), stop=(d == D_CHUNKS - 1))
                nc.vector.tensor_copy(
                    vt.rearrange("p t (h c) -> p t h c", h=HG_HEADS)[:, t, :, 64:128],
                    psv.rearrange("p (h c) -> p h c", h=HG_HEADS))

            # ---- Q^T, K^T col-tile 0 (heads 0-1)
            for w_sb, dst in ((wqh, qt), (wkh, kt)):
                for t4 in range(4):
                    ps = pq.tile([128, 512], F32, tag="qkps", bufs=2)
                    for d in range(D_CHUNKS):
                        nc.tensor.matmul(
                            ps,
                            lhsT=w_sb[:, d, 0:128],
                            rhs=xt[:, d, t4 * 512:(t4 + 1) * 512],
                            start=(d == 0), stop=(d == D_CHUNKS - 1))
                    nc.scalar.copy(dst[:, 0, t4 * 512:(t4 + 1) * 512], ps)

        # ---- attention (h-outer), QK col-tile 1 interleaved, out-proj tail
        with tc.psum_pool(name="pp2", bufs=1) as pa:

            def attention(h):
                hc, hr = h // 2, (h % 2) * 64   # col-tile, row offset in qt/kt
                for qh in range(QH):
                    ctp = pa.tile([128, QHW], F32, tag="ctp", bufs=2)
                    kmax = 8 * (qh + 1)
                    for t in range(kmax):
                        lo = max(0, t * 128 - qh * QHW)   # first visible local col
                        chp = lo // 512                    # first contributing chunk
                        sp = pa.tile([128, QHW], F32, tag="sp", bufs=2)
                        for ch in range(chp, 2):
                            nc.tensor.matmul(
                                sp[:, ch * 512:(ch + 1) * 512],
                                lhsT=kt[hr:hr + 64, hc, t * 128:(t + 1) * 128],
                                rhs=qt[hr:hr + 64, hc,
                                       qh * QHW + ch * 512:qh * QHW + (ch + 1) * 512],
                                start=True, stop=True)
                        pb = wp.tile([128, QHW], BF16, tag="pb", bufs=3)
                        nc.scalar.activation(pb[:, lo:QHW], sp[:, lo:QHW], AF.Exp)
                        if lo > chp * 512:
                            nc.vector.memset(pb[:, chp * 512:lo], 0.0)
                        if t >= 8 * qh:   # diagonal tile: mask the boundary block
                            nc.vector.tensor_mul(
                                pb[:, lo:lo + 128], pb[:, lo:lo + 128], cmask)
                        for ch in range(chp, 2):
                            nc.tensor.matmul(
                                ctp[:, ch * 512:(ch + 1) * 512],
                                lhsT=vt[:, t, h * 128:(h + 1) * 128],
                                rhs=pb[:, ch * 512:(ch + 1) * 512],
                                start=(t == 0),
                                stop=(t == 8 * qh + 4 * ch + 3))
                    # Detach the PSUM accumulator with ONE fast copy so the
                    # normalization chain never blocks later PV matmuls
                    # (PSUM-held stalls re-throttle the PE clock).
                    cst = wp.tile([128, QHW], F32, tag="cst", bufs=2)
                    nc.vector.tensor_copy(cst, ctp)
                    # normalize: ctx^T = ctx~^T * (1/denom), denom is row 0
                    # (custom-DVE recip needs a partition-0 SBUF input)
                    rec = wp.tile([1, QHW], F32, tag="rec", bufs=2)
                    rscr = wp.tile([1, QHW], F32, tag="rscr", bufs=2)
                    nc.vector.reciprocal_approx_accurate(rec, cst[0:1, :], rscr)
                    bcr = wp.tile([128, QHW], F32, tag="bcr", bufs=2)
                    nc.gpsimd.partition_broadcast(bcr, rec, channels=128)
                    nc.vector.tensor_mul(
                        ctxT[hr:hr + 64, hc, qh * QHW:(qh + 1) * QHW],
                        cst[64:128, :], bcr[64:128, :])

            attention(0)
            attention(1)

            # Q^T, K^T col-tile 1 (heads 2-3) — PE filler while heads 0-1's
            # exp-gated attention runs, keeps the PE clock unthrottled
            for w_sb, dst in ((wqh, qt), (wkh, kt)):
                for t4 in range(4):
                    ps1 = pa.tile([128, 512], F32, tag="sp", bufs=2)
                    for d in range(D_CHUNKS):
                        nc.tensor.matmul(
                            ps1,
                            lhsT=w_sb[:, d, 128:256],
                            rhs=xt[:, d, t4 * 512:(t4 + 1) * 512],
                            start=(d == 0), stop=(d == D_CHUNKS - 1))
                    nc.scalar.copy(dst[:, 1, t4 * 512:(t4 + 1) * 512], ps1)

            attention(2)
            attention(3)

            # ---- out-projection
            for t in range(TOK_TILES):
                osb = wp.tile([128, D], F32, tag="osb", bufs=3)
                for n in range(2):
                    pso = pa.tile([128, 512], F32, tag="sp", bufs=2)
                    for x in range(2):
                        nc.tensor.matmul(
                            pso,
                            lhsT=ctxT[:, x, t * 128:(t + 1) * 128],
                            rhs=wob[:, x, n * 512:(n + 1) * 512],
                            start=(x == 0), stop=(x == 1))
                    if n == 0:
                        nc.scalar.copy(osb[:, n * 512:(n + 1) * 512], pso)
                    else:
                        nc.vector.tensor_copy(osb[:, n * 512:(n + 1) * 512], pso)
                nc.sync.dma_start(out_d[t * 128:(t + 1) * 128, :], osb)


_PROGRAM = None


def _get_program():
    global _PROGRAM
    if _PROGRAM is None:
        _PROGRAM = _build_program()
    return _PROGRAM


def make_in_maps(X, Wq, Wk, Wv, Wo):
    X = np.asarray(X, dtype=np.float32)
    Wq = np.asarray(Wq, dtype=np.float32)
    Wk = np.asarray(Wk, dtype=np.float32)
    Wv = np.asarray(Wv, dtype=np.float32)
    Wo = np.asarray(Wo, dtype=np.float32)
    in_maps = []
    for core in range(NCORES):
        b, hg = core // 4, core % 4
        cs = slice(hg * CG, (hg + 1) * CG)
        in_maps.append({
            "X": np.ascontiguousarray(X[b]),
            "Wq": np.ascontiguousarray(Wq[:, cs]),
            "Wk": np.ascontiguousarray(Wk[:, cs]),
            "Wv": np.ascontiguousarray(Wv[:, cs]),
            "Wo": np.ascontiguousarray(Wo[cs, :]),
        })
    return in_maps


def combine_outputs(results, bo):
    bo = np.asarray(bo, dtype=np.float32)
    out = np.empty((B, S, D), dtype=np.float32)
    for b in range(B):
        acc = results[b * 4]["out"].copy()
        for hg in range(1, 4):
            acc += results[b * 4 + hg]["out"]
        out[b] = acc + bo[None, :]
    return out


def run(X, Wq, Wk, Wv, Wo, bo, **spmd_kwargs):
    nc = _get_program()
    in_maps = make_in_maps(X, Wq, Wk, Wv, Wo)
    res = run_bass_kernel_spmd(nc, in_maps, core_ids=list(range(NCORES)),
                               **spmd_kwargs)
    return combine_outputs(res.results, bo), res


def kernel(X, Wq, Wk, Wv, Wo, bo):
    out, _ = run(X, Wq, Wk, Wv, Wo, bo)
    return out


# revision 19
# speedup vs baseline: 1.2642x; 1.2642x over previous
"""Multi-head causal attention (B=2, S=2048, D=1024, H=16) on 8 TRN2 NeuronCores.

Sharding: batch x head-group.  Core i handles batch b = i//4 and head-group
hg = i%4 (4 heads = 256 projection columns).  Each core computes
  Q^T/K^T/V = proj(X_b) for its 256 columns, causal attention for its 4
  heads, and a partial output  ctx_slice @ Wo[256-row slice]  ->
  [2048, 1024] fp32 partial.  Host sums the 4 partials per batch and adds bo
  (standard tensor-parallel row-sharded out-projection unshard).

On-core algorithm (per core):
  - X^T tiles built once via PE transposes (fp16).
  - Q^T = Wq^T X^T-route: lhsT=Wq chunk, rhs=X^T  -> Q^T [cols, tok] (fp16)
  - scores computed TRANSPOSED: S^T[k, q] = K @ Q^T via lhsT=K^T slice,
    rhs=Q^T, so softmax's k-reduction lands on the partition axis where the
    PE (ones-column trick) does it for free.
  - softmax without row-max: scores are bounded (|s| < 70 << 88), exp in
    fp32 cannot overflow. P = exp(S^T) in bf16.
  - PV: lhsT = [V | 1] (ones col appended), rhs = P^T -> ctx~^T and the
    softmax denominator in one accumulation.  Normalize with
    reciprocal + gpsimd partition_broadcast + DVE multiply.
  - out-proj: lhsT = ctx^T (already transposed!), rhs = Wo rows (bf16).
Pre-softmax chain runs in fp16 (1 cyc/row on PE, 8x finer mantissa than
bf16), post-softmax in bf16.  Measured end-to-end rel err ~4e-3.
"""

import numpy as np

import concourse.bass as bass
import concourse.mybir as mybir
import concourse.tile as tile
from concourse import bacc
from concourse.bass_utils import run_bass_kernel_spmd
from concourse.masks import make_identity

F32 = mybir.dt.float32
F16 = mybir.dt.float16
BF16 = mybir.dt.bfloat16
AF = mybir.ActivationFunctionType

B, S, D = 2, 2048, 1024
H, HD = 16, 64
NCORES = 8
CG = 256            # projection columns per core (4 heads)
HG_HEADS = 4        # heads per core
TOK_TILES = S // 128   # 16
D_CHUNKS = D // 128    # 8
QH = 2              # q halves of 1024
QHW = 1024          # q-half width
KT = S // 128       # 16 k tiles


def _build_program():
    nc = bacc.Bacc("TRN2", target_bir_lowering=False, debug=False)

    x_d = nc.dram_tensor("X", [S, D], F32, kind="ExternalInput").ap()
    wq_d = nc.dram_tensor("Wq", [D, CG], F32, kind="ExternalInput").ap()
    wk_d = nc.dram_tensor("Wk", [D, CG], F32, kind="ExternalInput").ap()
    wv_d = nc.dram_tensor("Wv", [D, CG], F32, kind="ExternalInput").ap()
    wo_d = nc.dram_tensor("Wo", [CG, D], F32, kind="ExternalInput").ap()
    out_d = nc.dram_tensor("out", [S, D], F32, kind="ExternalOutput").ap()

    with tile.TileContext(nc) as tc:
        _emit(nc, tc, x_d, wq_d, wk_d, wv_d, wo_d, out_d)
    nc.compile()
    return nc


def _emit(nc, tc, x_d, wq_d, wk_d, wv_d, wo_d, out_d):
    with (
        tc.sbuf_pool(name="persist", bufs=1) as pp,
        tc.sbuf_pool(name="work", bufs=1) as wp,
    ):
        # ---- persistent SBUF tensors
        xt = pp.tile([128, D_CHUNKS, S], F16, name="xt")        # X^T  [dval, dchunk, tok]
        qt = pp.tile([128, 2, S], F16, name="qt")               # Q^T  [col, coltile, tok]
        kt = pp.tile([128, 2, S], F16, name="kt")
        vt = pp.tile([128, TOK_TILES, HG_HEADS * 128], BF16, name="vt")  # [1|pad|V] per head
        ctxT = pp.tile([128, 2, S], BF16, name="ctxT")
        wqh = pp.tile([128, D_CHUNKS, CG], F16, name="wqh")
        wkh = pp.tile([128, D_CHUNKS, CG], F16, name="wkh")
        wvh = pp.tile([128, D_CHUNKS, CG], F16, name="wvh")
        wob = pp.tile([128, 2, D], BF16, name="wob")
        ident = pp.tile([128, 128], F16, name="ident")
        cmask = pp.tile([128, 128], BF16, name="cmask")

        # ---- constants
        make_identity(nc, ident)
        # causal 0/1 mask for the diagonal 128x128 block of S^T[k, q]:
        # keep (1.0) where q >= k i.e. col >= partition.
        nc.gpsimd.memset(cmask, 1.0)
        nc.gpsimd.affine_select(
            out=cmask, in_=cmask, compare_op=mybir.AluOpType.is_ge,
            fill=0.0, base=0, pattern=[[1, 128]], channel_multiplier=-1,
        )
        # ones column of [1|pad|V] (ones FIRST so the softmax denominator
        # lands in PSUM row 0 — custom-DVE reciprocal needs a partition-0
        # input; V starts at col 64 so ctx rows are 32-aligned)
        ones_cols = vt.rearrange("p t (h c) -> p t h c", h=HG_HEADS)[:, :, :, 0:1]
        nc.gpsimd.memset(ones_cols, 1.0)

        # ---- X loads issued first so the transpose pipeline starts ASAP
        xs_tiles = []
        for t in range(TOK_TILES):
            xs = wp.tile([128, D], F32, tag="xs", bufs=4, name=f"xs{t}")
            nc.sync.dma_start(xs, x_d[t * 128:(t + 1) * 128, :])
            xs_tiles.append(xs)

        # ---- load + convert weights
        for w_dram, w_sb in ((wq_d, wqh), (wk_d, wkh), (wv_d, wvh)):
            wstage = wp.tile([128, D_CHUNKS, CG], F32, tag="wstage", bufs=2)
            nc.sync.dma_start(wstage, w_dram.rearrange("(dc p) c -> p dc c", p=128))
            nc.vector.tensor_copy(w_sb, wstage)
        wostage = wp.tile([128, 2, D], F32, tag="wstage", bufs=2)
        nc.sync.dma_start(wostage, wo_d.rearrange("(ct p) n -> p ct n", p=128))
        nc.vector.tensor_copy(wob, wostage)

        with tc.psum_pool(name="pp1", bufs=1) as pq:
            # ---- X fp16 convert, PE-transpose into xt
            for t in range(TOK_TILES):
                xh = wp.tile([128, D], F16, tag="xh", bufs=3)
                nc.vector.tensor_copy(xh, xs_tiles[t])
                for dp in range(2):
                    xtp = pq.tile([128, 512], F16, tag="xtp", bufs=2)
                    for dd in range(4):
                        d = dp * 4 + dd
                        nc.tensor.transpose(
                            xtp[:, dd * 128:(dd + 1) * 128],
                            xh[:, d * 128:(d + 1) * 128], ident)
                    nc.scalar.copy(
                        xt[:, dp * 4:(dp + 1) * 4, t * 128:(t + 1) * 128],
                        xtp.rearrange("p (dd c) -> p dd c", dd=4))

            # ---- V  (lhsT = X^T tile, rhs = Wv)
            for t in range(TOK_TILES):
                psv = pq.tile([128, CG], F32, tag="vps", bufs=2)
                for d in range(D_CHUNKS):
                    nc.tensor.matmul(
                        psv,
                        lhsT=xt[:, d, t * 128:(t + 1) * 128],
                        rhs=wvh[:, d, :],
                        start=(d == 0), stop=(d == D_CHUNKS - 1))
                nc.vector.tensor_copy(
                    vt.rearrange("p t (h c) -> p t h c", h=HG_HEADS)[:, t, :, 64:128],
                    psv.rearrange("p (h c) -> p h c", h=HG_HEADS))

            # ---- Q^T, K^T col-tile 0 (heads 0-1)
            for w_sb, dst in ((wqh, qt), (wkh, kt)):
                for t4 in range(4):
                    ps = pq.tile([128, 512], F32, tag="qkps", bufs=2)
                    for d in range(D_CHUNKS):
                        nc.tensor.matmul(
                            ps,
                            lhsT=w_sb[:, d, 0:128],
                            rhs=xt[:, d, t4 * 512:(t4 + 1) * 512],
                            start=(d == 0), stop=(d == D_CHUNKS - 1))
                    nc.scalar.copy(dst[:, 0, t4 * 512:(t4 + 1) * 512], ps)

        # ---- attention (h-outer), QK col-tile 1 interleaved, out-proj tail
        with tc.psum_pool(name="pp2", bufs=1) as pa:

            def attention(h):
                hc, hr = h // 2, (h % 2) * 64   # col-tile, row offset in qt/kt
                for qh in range(QH):
                    ctp = pa.tile([128, QHW], F32, tag="ctp", bufs=2)
                    kmax = 8 * (qh + 1)
                    for t in range(kmax):
                        lo = max(0, t * 128 - qh * QHW)   # first visible local col
                        chp = lo // 512                    # first contributing chunk
                        sp = pa.tile([128, QHW], F32, tag="sp", bufs=2)
                        for ch in range(chp, 2):
                            nc.tensor.matmul(
                                sp[:, ch * 512:(ch + 1) * 512],
                                lhsT=kt[hr:hr + 64, hc, t * 128:(t + 1) * 128],
                                rhs=qt[hr:hr + 64, hc,
                                       qh * QHW + ch * 512:qh * QHW + (ch + 1) * 512],
                                start=True, stop=True)
                        pb = wp.tile([128, QHW], BF16, tag="pb", bufs=3)
                        nc.scalar.activation(pb[:, lo:QHW], sp[:, lo:QHW], AF.Exp)
                        if lo > chp * 512:
                            nc.vector.memset(pb[:, chp * 512:lo], 0.0)
                        if t >= 8 * qh:   # diagonal tile: mask the boundary block
                            nc.vector.tensor_mul(
                                pb[:, lo:lo + 128], pb[:, lo:lo + 128], cmask)
                        for ch in range(chp, 2):
                            nc.tensor.matmul(
                                ctp[:, ch * 512:(ch + 1) * 512],
                                lhsT=vt[:, t, h * 128:(h + 1) * 128],
                                rhs=pb[:, ch * 512:(ch + 1) * 512],
                                start=(t == 0),
                                stop=(t == 8 * qh + 4 * ch + 3))
                    # Detach the PSUM accumulator with ONE fast copy so the
                    # normalization chain never blocks later PV matmuls
                    # (PSUM-held stalls re-throttle the PE clock).
                    cst = wp.tile([128, QHW], F32, tag="cst", bufs=2)
                    nc.vector.tensor_copy(cst, ctp)
                    # normalize: ctx^T = ctx~^T * (1/denom), denom is row 0
                    # (custom-DVE recip needs a partition-0 SBUF input)
                    rec = wp.tile([1, QHW], F32, tag="rec", bufs=2)
                    rscr = wp.tile([1, QHW], F32, tag="rscr", bufs=2)
                    nc.vector.reciprocal_approx_accurate(rec, cst[0:1, :], rscr)
                    bcr = wp.tile([128, QHW], F32, tag="bcr", bufs=2)
                    nc.gpsimd.partition_broadcast(bcr, rec, channels=128)
                    nc.vector.tensor_mul(
                        ctxT[hr:hr + 64, hc, qh * QHW:(qh + 1) * QHW],
                        cst[64:128, :], bcr[64:128, :])

            attention(0)
            attention(1)

            # Q^T, K^T col-tile 1 (heads 2-3) — PE filler while heads 0-1's
            # exp-gated attention runs, keeps the PE clock unthrottled
            for w_sb, dst in ((wqh, qt), (wkh, kt)):
                for t4 in range(4):
                    ps1 = pa.tile([128, 512], F32, tag="sp", bufs=2)
                    for d in range(D_CHUNKS):
                        nc.tensor.matmul(
                            ps1,
                            lhsT=w_sb[:, d, 128:256],
                            rhs=xt[:, d, t4 * 512:(t4 + 1) * 512],
                            start=(d == 0), stop=(d == D_CHUNKS - 1))
                    nc.scalar.copy(dst[:, 1, t4 * 512:(t4 + 1) * 512], ps1)

            attention(2)
            attention(3)

            # ---- out-projection
            for t in range(TOK_TILES):
                osb = wp.tile([128, D], F32, tag="osb", bufs=3)
                for n in range(2):
                    pso = pa.tile([128, 512], F32, tag="sp", bufs=2)
                    for x in range(2):
                        nc.tensor.matmul(
                            pso,
                            lhsT=ctxT[:, x, t * 128:(t + 1) * 128],
                            rhs=wob[:, x, n * 512:(n + 1) * 512],
                            start=(x == 0), stop=(x == 1))
                    if n == 0:
                        nc.scalar.copy(osb[:, n * 512:(n + 1) * 512], pso)
                    else:
                        nc.vector.tensor_copy(osb[:, n * 512:(n + 1) * 512], pso)
                nc.sync.dma_start(out_d[t * 128:(t + 1) * 128, :], osb)


_PROGRAM = None


def _get_program():
    global _PROGRAM
    if _PROGRAM is None:
        _PROGRAM = _build_program()
    return _PROGRAM


def make_in_maps(X, Wq, Wk, Wv, Wo):
    X = np.asarray(X, dtype=np.float32)
    Wq = np.asarray(Wq, dtype=np.float32)
    Wk = np.asarray(Wk, dtype=np.float32)
    Wv = np.asarray(Wv, dtype=np.float32)
    Wo = np.asarray(Wo, dtype=np.float32)
    in_maps = []
    for core in range(NCORES):
        b, hg = core // 4, core % 4
        cs = slice(hg * CG, (hg + 1) * CG)
        in_maps.append({
            "X": np.ascontiguousarray(X[b]),
            "Wq": np.ascontiguousarray(Wq[:, cs]),
            "Wk": np.ascontiguousarray(Wk[:, cs]),
            "Wv": np.ascontiguousarray(Wv[:, cs]),
            "Wo": np.ascontiguousarray(Wo[cs, :]),
        })
    return in_maps


def combine_outputs(results, bo):
    bo = np.asarray(bo, dtype=np.float32)
    out = np.empty((B, S, D), dtype=np.float32)
    for b in range(B):
        acc = results[b * 4]["out"].copy()
        for hg in range(1, 4):
            acc += results[b * 4 + hg]["out"]
        out[b] = acc + bo[None, :]
    return out


def run(X, Wq, Wk, Wv, Wo, bo, **spmd_kwargs):
    nc = _get_program()
    in_maps = make_in_maps(X, Wq, Wk, Wv, Wo)
    res = run_bass_kernel_spmd(nc, in_maps, core_ids=list(range(NCORES)),
                               **spmd_kwargs)
    return combine_outputs(res.results, bo), res


def kernel(X, Wq, Wk, Wv, Wo, bo):
    out, _ = run(X, Wq, Wk, Wv, Wo, bo)
    return out
